# revision 14
# baseline (speedup 1.0000x reference)
"""ChannelDropOut Trainium2 kernel.

Reference semantics (B,C,H,W = 64,512,32,32):
    gi        = X.mean(axis=(0,2,3))                      (C,)
    reward    = where(gi >= 0.5, 1.0, 0.0)
    new_alpha = alpha_param + reward
    new_beta  = beta_param + (1.0 - reward)
    keep_prob = Beta(alpha_param, beta_param)  sampled with jax key 42
    mask      = Bernoulli(keep_prob)                      (C,)
    out       = X * mask[None, :, None, None]

Device split: pure data parallel over batch across 8 NeuronCores. The
Beta/Bernoulli sampling is a tiny (C,)-sized op done host-side with jax CPU
(bit-identical to the reference, which cannot run on-neuron anyway since
jax.random.beta lowers to a while loop). The heavy work — one full read of X
for the per-channel sums plus the masked copy — runs on the cores.

The channel mask is known before the Bass program is built, so the program is
specialized to it: X is streamed through SBUF once, reduced per channel, and
only *kept* channel runs are DMA'd back out (out == X there). Dropped channels
are never written — run_bass_kernel_spmd zero-initializes output buffers on
both the native and the axon/PJRT path, so their zeros are free.

Layout: to get large contiguous DMA packets (the b-stride in (b,c,s) breaks
contiguity at 4KB otherwise), each SBUF partition is a (channel-group, batch)
pair: p = cg*8 + b with cg in 0..15 covering 32 consecutive channels. A tile's
free dim is then 8 consecutive channels x spatial = 32KB of contiguous DRAM
per partition row. Loads go on the sync HWDGE ring, stores on the scalar
HWDGE ring so the two streams don't serialize on one descriptor generator.
"""

import numpy as np

NCORES = 8
PT = 128  # SBUF partitions
CPG = 32  # consecutive channels per partition group (= C // 16)
TJ = 8  # channels per tile within a group (CPG // TJ tiles)
REWARD_VALUE = 1.0

_prog_cache: dict = {}

# test.py hooks: set TRACE=True before calling kernel() to profile; the
# resulting BassKernelResults lands in LAST_RESULTS.
TRACE = False
LAST_RESULTS = None


def _compute_mask(alpha, beta):
    """Replicate the reference's sampling exactly, on jax CPU."""
    import jax

    cpu = jax.devices("cpu")[0]
    with jax.default_device(cpu):
        k_beta, k_bern = jax.random.split(jax.random.key(42))
        keep_prob = jax.random.beta(
            k_beta, jax.numpy.asarray(alpha), jax.numpy.asarray(beta)
        )
        mask = jax.random.bernoulli(k_bern, keep_prob)
        return np.asarray(mask)  # bool (C,)


def _rects_per_tile(mask):
    """Cover each tile's kept (cg, j) bitmap with rectangles.

    Tile t's bitmap cell (cg, j) is channel 32*cg + TJ*t + j. A rectangle
    (cg0, cg1, j0, j1) becomes one store DMA spanning (cg1-cg0)*bc partitions
    and (j1-j0)*S contiguous elements per (cg, b) row. Greedy: per-cg maximal
    kept j-intervals, merged across consecutive cgs when identical.
    """
    C = mask.shape[0]
    ngrp = C // CPG
    nt = CPG // TJ
    per_tile = []
    for t in range(nt):
        rects = []
        active = {}  # (j0, j1) -> cg_start
        for cg in range(ngrp + 1):
            ivs = set()
            if cg < ngrp:
                j = 0
                while j < TJ:
                    if mask[cg * CPG + t * TJ + j]:
                        j0 = j
                        while j < TJ and mask[cg * CPG + t * TJ + j]:
                            j += 1
                        ivs.add((j0, j))
                    else:
                        j += 1
            for iv in [iv for iv in active if iv not in ivs]:
                rects.append((active.pop(iv), cg, iv[0], iv[1]))
            for iv in ivs:
                active.setdefault(iv, cg)
        per_tile.append(rects)
    return per_tile


def _build_program(bc, C, S, mask):
    import concourse.bacc as bacc
    import concourse.mybir as mybir
    from concourse import tile

    ngrp = C // CPG  # 16 partition groups
    nt = CPG // TJ  # tiles
    assert ngrp * bc == PT
    f32 = mybir.dt.float32
    nc = bacc.Bacc("TRN2", target_bir_lowering=False, debug=False, num_devices=NCORES)
    x = nc.dram_tensor("x", [bc, C, S], f32, kind="ExternalInput")
    out = nc.dram_tensor("out", [bc, C, S], f32, kind="ExternalOutput")
    gsum = nc.dram_tensor("gsum", [PT, CPG], f32, kind="ExternalOutput")

    per_tile_rects = _rects_per_tile(mask)

    # (cg, b, 32*S contiguous channels-x-spatial)
    xv = x.rearrange("b (cg r) s -> cg b (r s)", cg=ngrp)
    outv = out.rearrange("b (cg r) s -> cg b (r s)", cg=ngrp)

    with tile.TileContext(nc) as tc:
        with (
            tc.tile_pool(name="lbuf", bufs=3) as lpool,
            tc.tile_pool(name="accp", bufs=1) as accp,
        ):
            acc = accp.tile([PT, CPG], f32)
            for t in range(nt):
                tl = lpool.tile([PT, TJ, S], f32, tag="in")
                # loads on the sync HWDGE ring; the raw-X reduce (DVE) and the
                # kept-rectangle stores (scalar HWDGE ring) both only READ the
                # tile, so they run concurrently and the buffer recycles as
                # soon as both finish. Dropped channels are never written —
                # the output buffer is pre-zeroed by the runtime.
                nc.sync.dma_start(tl[:], xv[:, :, t * TJ * S : (t + 1) * TJ * S])
                nc.vector.reduce_sum(
                    acc[:, t * TJ : (t + 1) * TJ], tl[:], axis=mybir.AxisListType.X
                )
                for i, (cg0, cg1, j0, j1) in enumerate(per_tile_rects[t]):
                    dst = outv[
                        cg0:cg1, :, (t * TJ + j0) * S : (t * TJ + j1) * S
                    ]
                    eng = nc.scalar if i % 2 == 0 else nc.gpsimd
                    eng.dma_start(dst, tl[cg0 * bc : cg1 * bc, j0:j1, :])
            nc.gpsimd.dma_start(gsum[:, :], acc[:])
    nc.compile()
    return nc


def kernel(X, alpha_param, beta_param, current_epoch):
    from concourse.bass_utils import run_bass_kernel_spmd

    global LAST_RESULTS

    X = np.ascontiguousarray(np.asarray(X, dtype=np.float32))
    alpha = np.asarray(alpha_param, dtype=np.float32)
    beta = np.asarray(beta_param, dtype=np.float32)
    B, C, H, W = X.shape
    S = H * W
    bc = B // NCORES

    mask = _compute_mask(alpha, beta)

    key = (mask.tobytes(), X.shape)
    if key not in _prog_cache:
        _prog_cache[key] = _build_program(bc, C, S, mask)
    nc = _prog_cache[key]

    X3 = X.reshape(B, C, S)
    in_maps = [{"x": X3[i * bc : (i + 1) * bc]} for i in range(NCORES)]
    res = run_bass_kernel_spmd(nc, in_maps, core_ids=list(range(NCORES)), trace=TRACE)
    LAST_RESULTS = res

    out = np.concatenate([r["out"] for r in res.results], axis=0).reshape(B, C, H, W)
    # gsum[p, r]: p = cg*bc + b, r = in-group channel; sum cores and b
    g = np.sum([r["gsum"] for r in res.results], axis=0, dtype=np.float32)
    g = g.reshape(C // CPG, bc, CPG).sum(axis=1, dtype=np.float32)
    gi = g.reshape(C) / np.float32(B * S)

    reward = np.where(gi >= 0.5, np.float32(REWARD_VALUE), np.float32(0.0))
    new_alpha = alpha + reward
    new_beta = beta + (np.float32(REWARD_VALUE) - reward)
    mask_proba = mask.astype(np.float32).reshape(1, C, 1, 1)
    return out, mask_proba, new_alpha, new_beta


# revision 15
# speedup vs baseline: 1.1155x; 1.1155x over previous
"""ChannelDropOut Trainium2 kernel.

Reference semantics (B,C,H,W = 64,512,32,32):
    gi        = X.mean(axis=(0,2,3))                      (C,)
    reward    = where(gi >= 0.5, 1.0, 0.0)
    new_alpha = alpha_param + reward
    new_beta  = beta_param + (1.0 - reward)
    keep_prob = Beta(alpha_param, beta_param)  sampled with jax key 42
    mask      = Bernoulli(keep_prob)                      (C,)
    out       = X * mask[None, :, None, None]

Device split: pure data parallel over batch across 8 NeuronCores. The
Beta/Bernoulli sampling is a tiny (C,)-sized op done host-side with jax CPU
(bit-identical to the reference, which cannot run on-neuron anyway since
jax.random.beta lowers to a while loop). The heavy work — one full read of X
for the per-channel sums plus the masked copy — runs on the cores.

The channel mask is known before the Bass program is built, so the program is
specialized to it: X is streamed through SBUF once, reduced per channel, and
only *kept* channel runs are DMA'd back out (out == X there). Dropped channels
are never written — run_bass_kernel_spmd zero-initializes output buffers on
both the native and the axon/PJRT path, so their zeros are free.

Layout: to get large contiguous DMA packets (the b-stride in (b,c,s) breaks
contiguity at 4KB otherwise), each SBUF partition is a (channel-group, batch)
pair: p = cg*8 + b with cg in 0..15 covering 32 consecutive channels. A tile's
free dim is then 8 consecutive channels x spatial = 32KB of contiguous DRAM
per partition row. Loads go on the sync HWDGE ring, stores on the scalar
HWDGE ring so the two streams don't serialize on one descriptor generator.
"""

import numpy as np

NCORES = 8
PT = 128  # SBUF partitions
CPG = 32  # consecutive channels per partition group (= C // 16)
TJ = 8  # channels per tile within a group (CPG // TJ tiles)
REWARD_VALUE = 1.0

_prog_cache: dict = {}

# test.py hooks: set TRACE=True before calling kernel() to profile; the
# resulting BassKernelResults lands in LAST_RESULTS.
TRACE = False
LAST_RESULTS = None


def _compute_mask(alpha, beta):
    """Replicate the reference's sampling exactly, on jax CPU."""
    import jax

    cpu = jax.devices("cpu")[0]
    with jax.default_device(cpu):
        k_beta, k_bern = jax.random.split(jax.random.key(42))
        keep_prob = jax.random.beta(
            k_beta, jax.numpy.asarray(alpha), jax.numpy.asarray(beta)
        )
        mask = jax.random.bernoulli(k_bern, keep_prob)
        return np.asarray(mask)  # bool (C,)


def _rects_per_tile(mask):
    """Cover each tile's kept (cg, j) bitmap with rectangles.

    Tile t's bitmap cell (cg, j) is channel 32*cg + TJ*t + j. A rectangle
    (cg0, cg1, j0, j1) becomes one store DMA spanning (cg1-cg0)*bc partitions
    and (j1-j0)*S contiguous elements per (cg, b) row. Greedy: per-cg maximal
    kept j-intervals, merged across consecutive cgs when identical.
    """
    C = mask.shape[0]
    ngrp = C // CPG
    nt = CPG // TJ
    per_tile = []
    for t in range(nt):
        rects = []
        active = {}  # (j0, j1) -> cg_start
        for cg in range(ngrp + 1):
            ivs = set()
            if cg < ngrp:
                j = 0
                while j < TJ:
                    if mask[cg * CPG + t * TJ + j]:
                        j0 = j
                        while j < TJ and mask[cg * CPG + t * TJ + j]:
                            j += 1
                        ivs.add((j0, j))
                    else:
                        j += 1
            for iv in [iv for iv in active if iv not in ivs]:
                rects.append((active.pop(iv), cg, iv[0], iv[1]))
            for iv in ivs:
                active.setdefault(iv, cg)
        per_tile.append(rects)
    return per_tile


def _build_program(bc, C, S, mask):
    import concourse.bacc as bacc
    import concourse.mybir as mybir
    from concourse import tile

    ngrp = C // CPG  # 16 partition groups
    nt = CPG // TJ  # tiles
    assert ngrp * bc == PT
    f32 = mybir.dt.float32
    nc = bacc.Bacc("TRN2", target_bir_lowering=False, debug=False, num_devices=NCORES)
    x = nc.dram_tensor("x", [bc, C, S], f32, kind="ExternalInput")
    out = nc.dram_tensor("out", [bc, C, S], f32, kind="ExternalOutput")
    gsum = nc.dram_tensor("gsum", [PT, CPG], f32, kind="ExternalOutput")

    # mask value per (partition, in-group channel): p = cg*bc + b
    mvals = np.repeat(
        mask.astype(np.float32).reshape(ngrp, CPG), bc, axis=0
    )  # (128, CPG)
    mconst = nc.inline_tensor(mvals, name="maskvals")

    # (cg, b, 32*S contiguous channels-x-spatial)
    xv = x.rearrange("b (cg r) s -> cg b (r s)", cg=ngrp)
    outv = out.rearrange("b (cg r) s -> cg b (r s)", cg=ngrp)

    with tile.TileContext(nc) as tc:
        with (
            tc.tile_pool(name="lbuf", bufs=3) as lpool,
            tc.tile_pool(name="obuf", bufs=2) as opool,
            tc.tile_pool(name="accp", bufs=1) as accp,
        ):
            mt = accp.tile([PT, CPG], f32)
            nc.sync.dma_start(mt[:], mconst[:, :])
            acc = accp.tile([PT, CPG], f32)
            for t in range(nt):
                tl = lpool.tile([PT, TJ, S], f32, tag="in")
                tl2 = opool.tile([PT, TJ, S], f32, tag="out")
                # loads alone on the sync HWDGE ring (sustains 400+ GB/s with
                # 32KB rows); DVE does the raw-X reduce in parallel with the
                # ACT masking; the store is POSTED BY ACT right after its own
                # multiplies (same in-order queue, so no cross-engine handoff
                # can head-of-line block it) onto the scalar HWDGE ring.
                nc.sync.dma_start(tl[:], xv[:, :, t * TJ * S : (t + 1) * TJ * S])
                nc.vector.reduce_sum(
                    acc[:, t * TJ : (t + 1) * TJ], tl[:], axis=mybir.AxisListType.X
                )
                for j in range(TJ):
                    col = t * TJ + j
                    nc.scalar.activation(
                        tl2[:, j, :],
                        tl[:, j, :],
                        mybir.ActivationFunctionType.Copy,
                        scale=mt[:, col : col + 1],
                    )
                nc.scalar.dma_start(
                    outv[:, :, t * TJ * S : (t + 1) * TJ * S], tl2[:]
                )
            nc.sync.dma_start(gsum[:, :], acc[:])
    nc.compile()
    return nc


def kernel(X, alpha_param, beta_param, current_epoch):
    from concourse.bass_utils import run_bass_kernel_spmd

    global LAST_RESULTS

    X = np.ascontiguousarray(np.asarray(X, dtype=np.float32))
    alpha = np.asarray(alpha_param, dtype=np.float32)
    beta = np.asarray(beta_param, dtype=np.float32)
    B, C, H, W = X.shape
    S = H * W
    bc = B // NCORES

    mask = _compute_mask(alpha, beta)

    key = (mask.tobytes(), X.shape)
    if key not in _prog_cache:
        _prog_cache[key] = _build_program(bc, C, S, mask)
    nc = _prog_cache[key]

    X3 = X.reshape(B, C, S)
    in_maps = [{"x": X3[i * bc : (i + 1) * bc]} for i in range(NCORES)]
    res = run_bass_kernel_spmd(nc, in_maps, core_ids=list(range(NCORES)), trace=TRACE)
    LAST_RESULTS = res

    out = np.concatenate([r["out"] for r in res.results], axis=0).reshape(B, C, H, W)
    # gsum[p, r]: p = cg*bc + b, r = in-group channel; sum cores and b
    g = np.sum([r["gsum"] for r in res.results], axis=0, dtype=np.float32)
    g = g.reshape(C // CPG, bc, CPG).sum(axis=1, dtype=np.float32)
    gi = g.reshape(C) / np.float32(B * S)

    reward = np.where(gi >= 0.5, np.float32(REWARD_VALUE), np.float32(0.0))
    new_alpha = alpha + reward
    new_beta = beta + (np.float32(REWARD_VALUE) - reward)
    mask_proba = mask.astype(np.float32).reshape(1, C, 1, 1)
    return out, mask_proba, new_alpha, new_beta


# revision 16
# speedup vs baseline: 1.4044x; 1.2590x over previous
"""ChannelDropOut Trainium2 kernel.

Reference semantics (B,C,H,W = 64,512,32,32):
    gi        = X.mean(axis=(0,2,3))                      (C,)
    reward    = where(gi >= 0.5, 1.0, 0.0)
    new_alpha = alpha_param + reward
    new_beta  = beta_param + (1.0 - reward)
    keep_prob = Beta(alpha_param, beta_param)  sampled with jax key 42
    mask      = Bernoulli(keep_prob)                      (C,)
    out       = X * mask[None, :, None, None]

Device split: pure data parallel over batch across 8 NeuronCores. The
Beta/Bernoulli sampling is a tiny (C,)-sized op done host-side with jax CPU
(bit-identical to the reference, which cannot run on-neuron anyway since
jax.random.beta lowers to a while loop). The heavy work — one full read of X
for the per-channel sums plus the masked copy — runs on the cores.

The channel mask is known before the Bass program is built, so the program is
specialized to it: X is streamed through SBUF once, reduced per channel, and
only *kept* channel runs are DMA'd back out (out == X there). Dropped channels
are never written — run_bass_kernel_spmd zero-initializes output buffers on
both the native and the axon/PJRT path, so their zeros are free.

Layout: to get large contiguous DMA packets (the b-stride in (b,c,s) breaks
contiguity at 4KB otherwise), each SBUF partition is a (channel-group, batch)
pair: p = cg*8 + b with cg in 0..15 covering 32 consecutive channels. A tile's
free dim is then 8 consecutive channels x spatial = 32KB of contiguous DRAM
per partition row. Loads go on the sync HWDGE ring, stores on the scalar
HWDGE ring so the two streams don't serialize on one descriptor generator.
"""

import numpy as np

NCORES = 8
PT = 128  # SBUF partitions
CPG = 32  # consecutive channels per partition group (= C // 16)
TJ = 4  # channels per tile within a group (CPG // TJ tiles)
REWARD_VALUE = 1.0

_prog_cache: dict = {}

# test.py hooks: set TRACE=True before calling kernel() to profile; the
# resulting BassKernelResults lands in LAST_RESULTS.
TRACE = False
LAST_RESULTS = None


def _compute_mask(alpha, beta):
    """Replicate the reference's sampling exactly, on jax CPU."""
    import jax

    cpu = jax.devices("cpu")[0]
    with jax.default_device(cpu):
        k_beta, k_bern = jax.random.split(jax.random.key(42))
        keep_prob = jax.random.beta(
            k_beta, jax.numpy.asarray(alpha), jax.numpy.asarray(beta)
        )
        mask = jax.random.bernoulli(k_bern, keep_prob)
        return np.asarray(mask)  # bool (C,)


def _rects_per_tile(mask):
    """Cover each tile's kept (cg, j) bitmap with rectangles.

    Tile t's bitmap cell (cg, j) is channel 32*cg + TJ*t + j. A rectangle
    (cg0, cg1, j0, j1) becomes one store DMA spanning (cg1-cg0)*bc partitions
    and (j1-j0)*S contiguous elements per (cg, b) row. Greedy: per-cg maximal
    kept j-intervals, merged across consecutive cgs when identical.
    """
    C = mask.shape[0]
    ngrp = C // CPG
    nt = CPG // TJ
    per_tile = []
    for t in range(nt):
        rects = []
        active = {}  # (j0, j1) -> cg_start
        for cg in range(ngrp + 1):
            ivs = set()
            if cg < ngrp:
                j = 0
                while j < TJ:
                    if mask[cg * CPG + t * TJ + j]:
                        j0 = j
                        while j < TJ and mask[cg * CPG + t * TJ + j]:
                            j += 1
                        ivs.add((j0, j))
                    else:
                        j += 1
            for iv in [iv for iv in active if iv not in ivs]:
                rects.append((active.pop(iv), cg, iv[0], iv[1]))
            for iv in ivs:
                active.setdefault(iv, cg)
        per_tile.append(rects)
    return per_tile


def _build_program(bc, C, S, mask):
    import concourse.bacc as bacc
    import concourse.mybir as mybir
    from concourse import tile

    ngrp = C // CPG  # 16 partition groups
    nt = CPG // TJ  # tiles
    assert ngrp * bc == PT
    f32 = mybir.dt.float32
    nc = bacc.Bacc("TRN2", target_bir_lowering=False, debug=False, num_devices=NCORES)
    x = nc.dram_tensor("x", [bc, C, S], f32, kind="ExternalInput")
    out = nc.dram_tensor("out", [bc, C, S], f32, kind="ExternalOutput")
    gsum = nc.dram_tensor("gsum", [PT, CPG], f32, kind="ExternalOutput")

    # mask value per (partition, in-group channel): p = cg*bc + b
    mvals = np.repeat(
        mask.astype(np.float32).reshape(ngrp, CPG), bc, axis=0
    )  # (128, CPG)
    mconst = nc.inline_tensor(mvals, name="maskvals")

    # (cg, b, 32*S contiguous channels-x-spatial)
    xv = x.rearrange("b (cg r) s -> cg b (r s)", cg=ngrp)
    outv = out.rearrange("b (cg r) s -> cg b (r s)", cg=ngrp)

    with tile.TileContext(nc) as tc:
        with (
            tc.tile_pool(name="lbuf", bufs=4) as lpool,
            tc.tile_pool(name="obuf", bufs=3) as opool,
            tc.tile_pool(name="accp", bufs=1) as accp,
        ):
            mt = accp.tile([PT, CPG], f32)
            nc.sync.dma_start(mt[:], mconst[:, :])
            acc = accp.tile([PT, CPG], f32)
            for t in range(nt):
                tl = lpool.tile([PT, TJ, S], f32, tag="in")
                tl2 = opool.tile([PT, TJ, S], f32, tag="out")
                # loads alone on the sync HWDGE ring (sustains 400+ GB/s with
                # 32KB rows); DVE does the raw-X reduce in parallel with the
                # ACT masking; the store is POSTED BY ACT right after its own
                # multiplies (same in-order queue, so no cross-engine handoff
                # can head-of-line block it) onto the scalar HWDGE ring.
                nc.sync.dma_start(tl[:], xv[:, :, t * TJ * S : (t + 1) * TJ * S])
                nc.vector.reduce_sum(
                    acc[:, t * TJ : (t + 1) * TJ], tl[:], axis=mybir.AxisListType.X
                )
                for j in range(TJ):
                    col = t * TJ + j
                    nc.scalar.activation(
                        tl2[:, j, :],
                        tl[:, j, :],
                        mybir.ActivationFunctionType.Copy,
                        scale=mt[:, col : col + 1],
                    )
                nc.gpsimd.dma_start(
                    outv[:, :, t * TJ * S : (t + 1) * TJ * S], tl2[:]
                )
            nc.sync.dma_start(gsum[:, :], acc[:])
    nc.compile()
    return nc


def kernel(X, alpha_param, beta_param, current_epoch):
    from concourse.bass_utils import run_bass_kernel_spmd

    global LAST_RESULTS

    X = np.ascontiguousarray(np.asarray(X, dtype=np.float32))
    alpha = np.asarray(alpha_param, dtype=np.float32)
    beta = np.asarray(beta_param, dtype=np.float32)
    B, C, H, W = X.shape
    S = H * W
    bc = B // NCORES

    mask = _compute_mask(alpha, beta)

    key = (mask.tobytes(), X.shape)
    if key not in _prog_cache:
        _prog_cache[key] = _build_program(bc, C, S, mask)
    nc = _prog_cache[key]

    X3 = X.reshape(B, C, S)
    in_maps = [{"x": X3[i * bc : (i + 1) * bc]} for i in range(NCORES)]
    res = run_bass_kernel_spmd(nc, in_maps, core_ids=list(range(NCORES)), trace=TRACE)
    LAST_RESULTS = res

    out = np.concatenate([r["out"] for r in res.results], axis=0).reshape(B, C, H, W)
    # gsum[p, r]: p = cg*bc + b, r = in-group channel; sum cores and b
    g = np.sum([r["gsum"] for r in res.results], axis=0, dtype=np.float32)
    g = g.reshape(C // CPG, bc, CPG).sum(axis=1, dtype=np.float32)
    gi = g.reshape(C) / np.float32(B * S)

    reward = np.where(gi >= 0.5, np.float32(REWARD_VALUE), np.float32(0.0))
    new_alpha = alpha + reward
    new_beta = beta + (np.float32(REWARD_VALUE) - reward)
    mask_proba = mask.astype(np.float32).reshape(1, C, 1, 1)
    return out, mask_proba, new_alpha, new_beta


# revision 17
# speedup vs baseline: 1.4161x; 1.0084x over previous
"""ChannelDropOut Trainium2 kernel.

Reference semantics (B,C,H,W = 64,512,32,32):
    gi        = X.mean(axis=(0,2,3))                      (C,)
    reward    = where(gi >= 0.5, 1.0, 0.0)
    new_alpha = alpha_param + reward
    new_beta  = beta_param + (1.0 - reward)
    keep_prob = Beta(alpha_param, beta_param)  sampled with jax key 42
    mask      = Bernoulli(keep_prob)                      (C,)
    out       = X * mask[None, :, None, None]

Device split: pure data parallel over batch across 8 NeuronCores. The
Beta/Bernoulli sampling is a tiny (C,)-sized op done host-side with jax CPU
(bit-identical to the reference, which cannot run on-neuron anyway since
jax.random.beta lowers to a while loop). The heavy work — one full read of X
for the per-channel sums plus the masked copy — runs on the cores.

The channel mask is known before the Bass program is built, so the program is
specialized to it: X is streamed through SBUF once, reduced per channel, and
only *kept* channel runs are DMA'd back out (out == X there). Dropped channels
are never written — run_bass_kernel_spmd zero-initializes output buffers on
both the native and the axon/PJRT path, so their zeros are free.

Layout: to get large contiguous DMA packets (the b-stride in (b,c,s) breaks
contiguity at 4KB otherwise), each SBUF partition is a (channel-group, batch)
pair: p = cg*8 + b with cg in 0..15 covering 32 consecutive channels. A tile's
free dim is then 8 consecutive channels x spatial = 32KB of contiguous DRAM
per partition row. Loads go on the sync HWDGE ring, stores on the scalar
HWDGE ring so the two streams don't serialize on one descriptor generator.
"""

import numpy as np

NCORES = 8
PT = 128  # SBUF partitions
CPG = 32  # consecutive channels per partition group (= C // 16)
TJ = 2  # channels per tile within a group (CPG // TJ tiles)
REWARD_VALUE = 1.0

_prog_cache: dict = {}

# test.py hooks: set TRACE=True before calling kernel() to profile; the
# resulting BassKernelResults lands in LAST_RESULTS.
TRACE = False
LAST_RESULTS = None


def _compute_mask(alpha, beta):
    """Replicate the reference's sampling exactly, on jax CPU."""
    import jax

    cpu = jax.devices("cpu")[0]
    with jax.default_device(cpu):
        k_beta, k_bern = jax.random.split(jax.random.key(42))
        keep_prob = jax.random.beta(
            k_beta, jax.numpy.asarray(alpha), jax.numpy.asarray(beta)
        )
        mask = jax.random.bernoulli(k_bern, keep_prob)
        return np.asarray(mask)  # bool (C,)


def _rects_per_tile(mask):
    """Cover each tile's kept (cg, j) bitmap with rectangles.

    Tile t's bitmap cell (cg, j) is channel 32*cg + TJ*t + j. A rectangle
    (cg0, cg1, j0, j1) becomes one store DMA spanning (cg1-cg0)*bc partitions
    and (j1-j0)*S contiguous elements per (cg, b) row. Greedy: per-cg maximal
    kept j-intervals, merged across consecutive cgs when identical.
    """
    C = mask.shape[0]
    ngrp = C // CPG
    nt = CPG // TJ
    per_tile = []
    for t in range(nt):
        rects = []
        active = {}  # (j0, j1) -> cg_start
        for cg in range(ngrp + 1):
            ivs = set()
            if cg < ngrp:
                j = 0
                while j < TJ:
                    if mask[cg * CPG + t * TJ + j]:
                        j0 = j
                        while j < TJ and mask[cg * CPG + t * TJ + j]:
                            j += 1
                        ivs.add((j0, j))
                    else:
                        j += 1
            for iv in [iv for iv in active if iv not in ivs]:
                rects.append((active.pop(iv), cg, iv[0], iv[1]))
            for iv in ivs:
                active.setdefault(iv, cg)
        per_tile.append(rects)
    return per_tile


def _build_program(bc, C, S, mask):
    import concourse.bacc as bacc
    import concourse.mybir as mybir
    from concourse import tile

    ngrp = C // CPG  # 16 partition groups
    nt = CPG // TJ  # tiles
    assert ngrp * bc == PT
    f32 = mybir.dt.float32
    nc = bacc.Bacc("TRN2", target_bir_lowering=False, debug=False, num_devices=NCORES)
    x = nc.dram_tensor("x", [bc, C, S], f32, kind="ExternalInput")
    out = nc.dram_tensor("out", [bc, C, S], f32, kind="ExternalOutput")
    gsum = nc.dram_tensor("gsum", [PT, CPG], f32, kind="ExternalOutput")

    # mask value per (partition, in-group channel): p = cg*bc + b
    mvals = np.repeat(
        mask.astype(np.float32).reshape(ngrp, CPG), bc, axis=0
    )  # (128, CPG)
    mconst = nc.inline_tensor(mvals, name="maskvals")

    # (cg, b, 32*S contiguous channels-x-spatial)
    xv = x.rearrange("b (cg r) s -> cg b (r s)", cg=ngrp)
    outv = out.rearrange("b (cg r) s -> cg b (r s)", cg=ngrp)

    with tile.TileContext(nc) as tc:
        with (
            tc.tile_pool(name="lbuf", bufs=6) as lpool,
            tc.tile_pool(name="obuf", bufs=4) as opool,
            tc.tile_pool(name="accp", bufs=1) as accp,
        ):
            mt = accp.tile([PT, CPG], f32)
            nc.sync.dma_start(mt[:], mconst[:, :])
            acc = accp.tile([PT, CPG], f32)
            for t in range(nt):
                tl = lpool.tile([PT, TJ, S], f32, tag="in")
                tl2 = opool.tile([PT, TJ, S], f32, tag="out")
                # loads alone on the sync HWDGE ring (sustains 400+ GB/s with
                # 32KB rows); DVE does the raw-X reduce in parallel with the
                # ACT masking; the store is POSTED BY ACT right after its own
                # multiplies (same in-order queue, so no cross-engine handoff
                # can head-of-line block it) onto the scalar HWDGE ring.
                nc.sync.dma_start(tl[:], xv[:, :, t * TJ * S : (t + 1) * TJ * S])
                nc.vector.reduce_sum(
                    acc[:, t * TJ : (t + 1) * TJ], tl[:], axis=mybir.AxisListType.X
                )
                for j in range(TJ):
                    col = t * TJ + j
                    nc.scalar.activation(
                        tl2[:, j, :],
                        tl[:, j, :],
                        mybir.ActivationFunctionType.Copy,
                        scale=mt[:, col : col + 1],
                    )
                nc.gpsimd.dma_start(
                    outv[:, :, t * TJ * S : (t + 1) * TJ * S], tl2[:]
                )
            nc.sync.dma_start(gsum[:, :], acc[:])
    nc.compile()
    return nc


def kernel(X, alpha_param, beta_param, current_epoch):
    from concourse.bass_utils import run_bass_kernel_spmd

    global LAST_RESULTS

    X = np.ascontiguousarray(np.asarray(X, dtype=np.float32))
    alpha = np.asarray(alpha_param, dtype=np.float32)
    beta = np.asarray(beta_param, dtype=np.float32)
    B, C, H, W = X.shape
    S = H * W
    bc = B // NCORES

    mask = _compute_mask(alpha, beta)

    key = (mask.tobytes(), X.shape)
    if key not in _prog_cache:
        _prog_cache[key] = _build_program(bc, C, S, mask)
    nc = _prog_cache[key]

    X3 = X.reshape(B, C, S)
    in_maps = [{"x": X3[i * bc : (i + 1) * bc]} for i in range(NCORES)]
    res = run_bass_kernel_spmd(nc, in_maps, core_ids=list(range(NCORES)), trace=TRACE)
    LAST_RESULTS = res

    out = np.concatenate([r["out"] for r in res.results], axis=0).reshape(B, C, H, W)
    # gsum[p, r]: p = cg*bc + b, r = in-group channel; sum cores and b
    g = np.sum([r["gsum"] for r in res.results], axis=0, dtype=np.float32)
    g = g.reshape(C // CPG, bc, CPG).sum(axis=1, dtype=np.float32)
    gi = g.reshape(C) / np.float32(B * S)

    reward = np.where(gi >= 0.5, np.float32(REWARD_VALUE), np.float32(0.0))
    new_alpha = alpha + reward
    new_beta = beta + (np.float32(REWARD_VALUE) - reward)
    mask_proba = mask.astype(np.float32).reshape(1, C, 1, 1)
    return out, mask_proba, new_alpha, new_beta


# revision 18
# speedup vs baseline: 1.4326x; 1.0117x over previous
"""ChannelDropOut Trainium2 kernel.

Reference semantics (B,C,H,W = 64,512,32,32):
    gi        = X.mean(axis=(0,2,3))                      (C,)
    reward    = where(gi >= 0.5, 1.0, 0.0)
    new_alpha = alpha_param + reward
    new_beta  = beta_param + (1.0 - reward)
    keep_prob = Beta(alpha_param, beta_param)  sampled with jax key 42
    mask      = Bernoulli(keep_prob)                      (C,)
    out       = X * mask[None, :, None, None]

Device split: pure data parallel over batch across 8 NeuronCores. The
Beta/Bernoulli sampling is a tiny (C,)-sized op done host-side with jax CPU
(bit-identical to the reference, which cannot run on-neuron anyway since
jax.random.beta lowers to a while loop). The heavy work — one full read of X
for the per-channel sums plus the masked copy — runs on the cores.

The channel mask is known before the Bass program is built, so the program is
specialized to it: X is streamed through SBUF once, reduced per channel, and
only *kept* channel runs are DMA'd back out (out == X there). Dropped channels
are never written — run_bass_kernel_spmd zero-initializes output buffers on
both the native and the axon/PJRT path, so their zeros are free.

Layout: to get large contiguous DMA packets (the b-stride in (b,c,s) breaks
contiguity at 4KB otherwise), each SBUF partition is a (channel-group, batch)
pair: p = cg*8 + b with cg in 0..15 covering 32 consecutive channels. A tile's
free dim is then 8 consecutive channels x spatial = 32KB of contiguous DRAM
per partition row. Loads go on the sync HWDGE ring, stores on the scalar
HWDGE ring so the two streams don't serialize on one descriptor generator.
"""

import numpy as np

NCORES = 8
PT = 128  # SBUF partitions
CPG = 32  # consecutive channels per partition group (= C // 16)
TJ = 2  # channels per tile within a group (CPG // TJ tiles)
REWARD_VALUE = 1.0

_prog_cache: dict = {}

# test.py hooks: set TRACE=True before calling kernel() to profile; the
# resulting BassKernelResults lands in LAST_RESULTS.
TRACE = False
LAST_RESULTS = None


def _compute_mask(alpha, beta):
    """Replicate the reference's sampling exactly, on jax CPU."""
    import jax

    cpu = jax.devices("cpu")[0]
    with jax.default_device(cpu):
        k_beta, k_bern = jax.random.split(jax.random.key(42))
        keep_prob = jax.random.beta(
            k_beta, jax.numpy.asarray(alpha), jax.numpy.asarray(beta)
        )
        mask = jax.random.bernoulli(k_bern, keep_prob)
        return np.asarray(mask)  # bool (C,)


def _rects_per_tile(mask):
    """Cover each tile's kept (cg, j) bitmap with rectangles.

    Tile t's bitmap cell (cg, j) is channel 32*cg + TJ*t + j. A rectangle
    (cg0, cg1, j0, j1) becomes one store DMA spanning (cg1-cg0)*bc partitions
    and (j1-j0)*S contiguous elements per (cg, b) row. Greedy: per-cg maximal
    kept j-intervals, merged across consecutive cgs when identical.
    """
    C = mask.shape[0]
    ngrp = C // CPG
    nt = CPG // TJ
    per_tile = []
    for t in range(nt):
        rects = []
        active = {}  # (j0, j1) -> cg_start
        for cg in range(ngrp + 1):
            ivs = set()
            if cg < ngrp:
                j = 0
                while j < TJ:
                    if mask[cg * CPG + t * TJ + j]:
                        j0 = j
                        while j < TJ and mask[cg * CPG + t * TJ + j]:
                            j += 1
                        ivs.add((j0, j))
                    else:
                        j += 1
            for iv in [iv for iv in active if iv not in ivs]:
                rects.append((active.pop(iv), cg, iv[0], iv[1]))
            for iv in ivs:
                active.setdefault(iv, cg)
        per_tile.append(rects)
    return per_tile


def _build_program(bc, C, S, mask):
    import concourse.bacc as bacc
    import concourse.mybir as mybir
    from concourse import tile

    ngrp = C // CPG  # 16 partition groups
    nt = CPG // TJ  # tiles
    assert ngrp * bc == PT
    f32 = mybir.dt.float32
    nc = bacc.Bacc("TRN2", target_bir_lowering=False, debug=False, num_devices=NCORES)
    x = nc.dram_tensor("x", [bc, C, S], f32, kind="ExternalInput")
    out = nc.dram_tensor("out", [bc, C, S], f32, kind="ExternalOutput")
    gsum = nc.dram_tensor("gsum", [PT, CPG], f32, kind="ExternalOutput")

    # mask value per (partition, in-group channel): p = cg*bc + b
    mvals = np.repeat(
        mask.astype(np.float32).reshape(ngrp, CPG), bc, axis=0
    )  # (128, CPG)
    mconst = nc.inline_tensor(mvals, name="maskvals")

    # (cg, b, 32*S contiguous channels-x-spatial)
    xv = x.rearrange("b (cg r) s -> cg b (r s)", cg=ngrp)
    outv = out.rearrange("b (cg r) s -> cg b (r s)", cg=ngrp)

    with tile.TileContext(nc) as tc:
        with (
            tc.tile_pool(name="lbuf", bufs=6) as lpool,
            tc.tile_pool(name="obuf", bufs=4) as opool,
            tc.tile_pool(name="accp", bufs=1) as accp,
        ):
            mt = accp.tile([PT, CPG], f32)
            nc.sync.dma_start(mt[:], mconst[:, :])
            acc = accp.tile([PT, CPG], f32)
            for t in range(nt):
                tl = lpool.tile([PT, TJ, S], f32, tag="in")
                tl2 = opool.tile([PT, TJ, S], f32, tag="out")
                # loads alone on the sync HWDGE ring (sustains 400+ GB/s with
                # 32KB rows); DVE does the raw-X reduce in parallel with the
                # ACT masking; the store is POSTED BY ACT right after its own
                # multiplies (same in-order queue, so no cross-engine handoff
                # can head-of-line block it) onto the scalar HWDGE ring.
                nc.sync.dma_start(tl[:], xv[:, :, t * TJ * S : (t + 1) * TJ * S])
                nc.vector.reduce_sum(
                    acc[:, t * TJ : (t + 1) * TJ], tl[:], axis=mybir.AxisListType.X
                )
                for j in range(TJ):
                    col = t * TJ + j
                    nc.scalar.activation(
                        tl2[:, j, :],
                        tl[:, j, :],
                        mybir.ActivationFunctionType.Copy,
                        scale=mt[:, col : col + 1],
                    )
                st_eng = nc.scalar if t % 2 == 0 else nc.gpsimd
                st_eng.dma_start(
                    outv[:, :, t * TJ * S : (t + 1) * TJ * S], tl2[:]
                )
            nc.sync.dma_start(gsum[:, :], acc[:])
    nc.compile()
    return nc


def kernel(X, alpha_param, beta_param, current_epoch):
    from concourse.bass_utils import run_bass_kernel_spmd

    global LAST_RESULTS

    X = np.ascontiguousarray(np.asarray(X, dtype=np.float32))
    alpha = np.asarray(alpha_param, dtype=np.float32)
    beta = np.asarray(beta_param, dtype=np.float32)
    B, C, H, W = X.shape
    S = H * W
    bc = B // NCORES

    mask = _compute_mask(alpha, beta)

    key = (mask.tobytes(), X.shape)
    if key not in _prog_cache:
        _prog_cache[key] = _build_program(bc, C, S, mask)
    nc = _prog_cache[key]

    X3 = X.reshape(B, C, S)
    in_maps = [{"x": X3[i * bc : (i + 1) * bc]} for i in range(NCORES)]
    res = run_bass_kernel_spmd(nc, in_maps, core_ids=list(range(NCORES)), trace=TRACE)
    LAST_RESULTS = res

    out = np.concatenate([r["out"] for r in res.results], axis=0).reshape(B, C, H, W)
    # gsum[p, r]: p = cg*bc + b, r = in-group channel; sum cores and b
    g = np.sum([r["gsum"] for r in res.results], axis=0, dtype=np.float32)
    g = g.reshape(C // CPG, bc, CPG).sum(axis=1, dtype=np.float32)
    gi = g.reshape(C) / np.float32(B * S)

    reward = np.where(gi >= 0.5, np.float32(REWARD_VALUE), np.float32(0.0))
    new_alpha = alpha + reward
    new_beta = beta + (np.float32(REWARD_VALUE) - reward)
    mask_proba = mask.astype(np.float32).reshape(1, C, 1, 1)
    return out, mask_proba, new_alpha, new_beta
